# revision 36
# baseline (speedup 1.0000x reference)
"""Dir_Encoder_GCN v4: bf16 streaming, chunked AllGather overlap, dma_gather.

Per core (nodes dst-sharded; self-loops materialized as edges, weight 1):

- Windows of <=128 nodes and <=TT*128 edges; TT=18 tiles/window split as
  tiles 0..8 = edges whose SOURCE lies in chunk A (source windows < WA on
  the source's core), tiles 9..17 = chunk-B sources.
- L1: x rows streamed in edge order (bf16, one descriptor per partition
  row); matmul(lhsT=x_tile, rhs=M) accumulates [F, slot] in PSUM; W1
  applies from it. M = one-hot(slot) * w_e * dinv[src] built by
  is_equal+mult from a bf16 iota (2x DVE mode); a fraction of M builds
  runs on the Pool engine (tunable).
- dinv[src] per edge from a bf16 ELL of source in-edge weights, reduced +
  rsqrt in a prologue (activation tables load once: Sqrt -> Exp -> Ln).
- t2 = dinv*(h@W2) rows PADDED to 128 bf16 (=256B, dma_gather alignment),
  staged per chunk; AllGather chunk A fires mid-L1 (overlaps remaining L1
  windows), chunk B at L1 end (overlaps L2 pass A). Two tables keep
  row indices < 32k for int16 dma_gather indices.
- L2 two passes (A-tiles then B-tiles): dma_gather in 1024-index chunks
  (8 tiles) decoupled from windows, lazily emitted ahead of use; pass A
  stashes partial sums, pass B combines; softplus finishes in bulk.

build_problem(..., reps=N) repeats the body N times inside one NEFF for
slope-based timing.
"""

import os
import sys

if "/opt/trn_rl_repo" not in sys.path:
    sys.path.insert(0, "/opt/trn_rl_repo")

import numpy as np

N_NODES = 50000
NCORES = 8
P = 128
TA = 9
TB = 9
TT = TA + TB  # 18
M_POOL_MOD = int(os.environ.get("M_POOL_MOD", "3"))  # every k-th M build on Pool; 0=off
KVARIANT = os.environ.get("KVARIANT", "full")  # full | l1cc | l1g (perf bisection)


def _pack2(cA, cB, margin=64):
    """Greedy contiguous windows: <=128 nodes, class caps minus a margin
    (the margin absorbs chunk-flag drift across packing iterations)."""
    windows = []
    lo = 0
    curA = 0
    curB = 0
    capA = TA * P - margin
    capB = TB * P - margin
    n = len(cA)
    hi = 0
    while hi < n:
        a, b = cA[hi], cB[hi]
        if (hi - lo) >= P or curA + a > capA or curB + b > capB:
            windows.append((lo, hi))
            lo = hi
            curA = 0
            curB = 0
        curA += a
        curB += b
        hi += 1
    windows.append((lo, hi))
    return windows


def build_problem(x, edge_index, edge_weight, W1, b1, W2, b2, reps=1):
    import concourse.bacc as bacc
    import concourse.tile as tile
    from concourse import bass, mybir

    bf16 = mybir.dt.bfloat16
    np_bf16 = mybir.dt.np(bf16)

    x = np.asarray(x, dtype=np.float32)
    edge_index = np.asarray(edge_index)
    ew = np.asarray(edge_weight, dtype=np.float32)
    W1 = np.asarray(W1, dtype=np.float32)
    b1 = np.asarray(b1, dtype=np.float32)
    W2 = np.asarray(W2, dtype=np.float32)
    b2 = np.asarray(b2, dtype=np.float32)

    n = x.shape[0]
    F = x.shape[1]          # 128
    H = W1.shape[1]         # 128
    FO = W2.shape[1]        # 64
    per_core_n = (n + NCORES - 1) // NCORES

    # real edges sorted by dst (source in-weight lists for the ELL)
    src_r = edge_index[0].astype(np.int64)
    dst_r = edge_index[1].astype(np.int64)
    order_r = np.argsort(dst_r, kind="stable")
    rs, rd, rw = src_r[order_r], dst_r[order_r], ew[order_r]
    r_e0 = np.searchsorted(rd, np.arange(n + 1))
    real_indeg = np.diff(r_e0)
    KD = max(1, int(real_indeg.max()))

    # all edges incl. self-loops (weight 1), sorted by dst
    src = np.concatenate([rs, np.arange(n, dtype=np.int64)])
    dst = np.concatenate([rd, np.arange(n, dtype=np.int64)])
    wgt = np.concatenate([rw, np.ones(n, dtype=np.float32)])
    order = np.argsort(dst, kind="stable")
    s_d, d_d, w_d = src[order], dst[order], wgt[order]
    node_e0 = np.searchsorted(d_d, np.arange(n + 1))
    indeg = np.diff(node_e0)

    core_lims = [(c * per_core_n, min((c + 1) * per_core_n, n)) for c in range(NCORES)]

    # --- window packing with A/B class caps; chunk flags depend on the
    # packing (source window < WA), so iterate with a growing safety
    # margin until the packing satisfies the full caps under the flags it
    # itself induces ---
    cA = indeg // 2
    cB = indeg - cA
    margin = 64
    core_windows = [_pack2(cA[lo:hi], cB[lo:hi], margin) for lo, hi in core_lims]
    for _it in range(20):
        NWIN = max(len(w) for w in core_windows)
        WA = min((NWIN + 1) // 2, 31)
        win_of_node = np.zeros(n, dtype=np.int64)
        for c, (n_lo, _) in enumerate(core_lims):
            for wi, (lo, hi) in enumerate(core_windows[c]):
                win_of_node[n_lo + lo : n_lo + hi] = wi
        isA_it = win_of_node < WA
        cA = np.bincount(d_d[isA_it[s_d]], minlength=n).astype(np.int64)
        cB = indeg - cA
        viol = 0
        for c, (n_lo, n_hi) in enumerate(core_lims):
            for lo, hi in core_windows[c]:
                if (
                    int(cA[n_lo + lo : n_lo + hi].sum()) > TA * P
                    or int(cB[n_lo + lo : n_lo + hi].sum()) > TB * P
                ):
                    viol += 1
        if viol == 0:
            break
        margin += 32
        core_windows = [_pack2(cA[lo:hi], cB[lo:hi], margin) for lo, hi in core_lims]
    else:
        raise RuntimeError("window packing did not converge")
    NWIN = max(len(w) for w in core_windows)
    WA = min((NWIN + 1) // 2, 31)
    WB = NWIN - WA
    assert WB <= 31, (NWIN, WA, WB)
    # final chunk flags from the final packing; verify full caps under them
    win_of_node = np.zeros(n, dtype=np.int64)
    for c, (n_lo, _) in enumerate(core_lims):
        for wi, (lo, hi) in enumerate(core_windows[c]):
            win_of_node[n_lo + lo : n_lo + hi] = wi
    isA_f = win_of_node < WA
    cA = np.bincount(d_d[isA_f[s_d]], minlength=n).astype(np.int64)
    cB = indeg - cA
    for c, (n_lo, n_hi) in enumerate(core_lims):
        for lo, hi in core_windows[c]:
            a = int(cA[n_lo + lo : n_lo + hi].sum())
            b = int(cB[n_lo + lo : n_lo + hi].sum())
            assert a <= TA * P and b <= TB * P and (hi - lo) <= P, (c, lo, hi, a, b)

    SH = NWIN * P
    TTOT = NWIN * TT
    RA = NCORES * WA * P  # chunk-A table rows
    RB = NCORES * WB * P
    assert RA <= 32640 and RB <= 32640, (RA, RB)

    # global rows
    row_of_node = np.zeros(n, dtype=np.int64)   # output rows: c*SH + wi*P + s
    crow_of_node = np.zeros(n, dtype=np.int64)  # chunk-local t2 rows
    win_of_node = np.zeros(n, dtype=np.int64)
    for c, (n_lo, _) in enumerate(core_lims):
        for wi, (lo, hi) in enumerate(core_windows[c]):
            ids = np.arange(lo, hi)
            row_of_node[n_lo + ids] = c * SH + wi * P + (ids - lo)
            win_of_node[n_lo + ids] = wi
            if wi < WA:
                crow_of_node[n_lo + ids] = c * (WA * P) + wi * P + (ids - lo)
            else:
                crow_of_node[n_lo + ids] = c * (WB * P) + (wi - WA) * P + (ids - lo)
    isA = win_of_node < WA

    NTA = NWIN * TA  # flat A tiles per core
    NTB = NWIN * TB
    CIA = NTA * P // 16  # idx cols
    CIB = NTB * P // 16

    xbf = x.astype(np_bf16)
    in_maps = []
    for c, (n_lo, n_hi) in enumerate(core_lims):
        wins = core_windows[c]

        xe = np.zeros((NWIN * P, TT * F), dtype=np_bf16)
        ell = np.zeros((P, TTOT * KD), dtype=np_bf16)
        slots = np.full((P, TTOT), -1.0, dtype=np.float32)
        wraw = np.zeros((P, TTOT), dtype=np.float32)
        idxA_flat = np.zeros(NTA * P, dtype=np.int64)
        idxB_flat = np.zeros(NTB * P, dtype=np.int64)
        wdegn = np.zeros((P, NWIN * KD), dtype=np_bf16)

        for wi, (lo, hi) in enumerate(wins):
            a0 = node_e0[n_lo + lo]
            b0 = node_e0[n_lo + hi]
            wsrc = s_d[a0:b0]
            wslot = (d_d[a0:b0] - n_lo - lo).astype(np.int64)
            ww = w_d[a0:b0]
            eA = isA[wsrc]
            # per-edge tile (t_g global col) and partition (pp)
            t_g = np.zeros(len(wsrc), dtype=np.int64)
            pp = np.zeros(len(wsrc), dtype=np.int64)
            jA = np.nonzero(eA)[0]
            jB = np.nonzero(~eA)[0]
            t_g[jA] = wi * TT + (np.arange(len(jA)) // P)
            pp[jA] = np.arange(len(jA)) % P
            t_g[jB] = wi * TT + TA + (np.arange(len(jB)) // P)
            pp[jB] = np.arange(len(jB)) % P

            slots[pp, t_g] = wslot.astype(np.float32)
            wraw[pp, t_g] = ww
            xe.reshape(NWIN * P, TT, F)[wi * P + pp, t_g - wi * TT] = xbf[wsrc]
            # gather index streams (chunk-local rows)
            tlA = t_g[jA] - wi * TT
            idxA_flat[(wi * TA + tlA) * P + pp[jA]] = crow_of_node[wsrc[jA]]
            tlB = t_g[jB] - wi * TT - TA
            idxB_flat[(wi * TB + tlB) * P + pp[jB]] = crow_of_node[wsrc[jB]]
            # per-edge source real-in-weight ELL
            lens = real_indeg[wsrc]
            tot = int(lens.sum())
            if tot:
                rep = np.repeat(np.arange(len(wsrc)), lens)
                offs = np.arange(tot) - np.repeat(np.cumsum(lens) - lens, lens)
                vals = rw[np.repeat(r_e0[wsrc], lens) + offs]
                ell.reshape(-1)[
                    pp[rep] * (TTOT * KD) + t_g[rep] * KD + offs
                ] = vals
            # own-node real-in-weight ELL for dinv_dst
            ids = np.arange(lo, hi)
            nl = real_indeg[n_lo + ids]
            ntot = int(nl.sum())
            if ntot:
                nrep = np.repeat(ids - lo, nl)
                noffs = np.arange(ntot) - np.repeat(np.cumsum(nl) - nl, nl)
                nvals = rw[np.repeat(r_e0[n_lo + ids], nl) + noffs]
                wdegn.reshape(-1)[
                    nrep * (NWIN * KD) + wi * KD + noffs
                ] = nvals

        assert idxA_flat.max() < RA and idxB_flat.max() < RB
        idxA16 = np.tile(idxA_flat.reshape(-1, 16).T, (8, 1)).astype(np.int16)
        idxB16 = np.tile(idxB_flat.reshape(-1, 16).T, (8, 1)).astype(np.int16)

        in_maps.append(
            {
                "xe": xe,
                "ell": ell,
                "slots": slots,
                "wraw": wraw,
                "idxA": idxA16,
                "idxB": idxB16,
                "wdegn": wdegn,
                "iotar": np.tile(np.arange(P, dtype=np_bf16), (P, 1)),
                "w1": W1.astype(np_bf16),
                "w2": W2.astype(np_bf16),
                "b1b": np.tile(b1[None, :], (P, 1)).astype(np.float32),
                "b2b": np.tile(b2[None, :], (P, 1)).astype(np.float32),
                "ident": np.eye(P, dtype=np_bf16),
            }
        )

    # ---------------- device program ----------------
    nc = bacc.Bacc(
        "TRN2", target_bir_lowering=False, debug=False, num_devices=NCORES,
        num_swdge_queues=4,
    )

    xe_d = nc.dram_tensor("xe", [NWIN * P, TT * F], bf16, kind="ExternalInput")
    ell_d = nc.dram_tensor("ell", [P, TTOT * KD], bf16, kind="ExternalInput")
    slots_d = nc.dram_tensor("slots", [P, TTOT], mybir.dt.float32, kind="ExternalInput")
    wraw_d = nc.dram_tensor("wraw", [P, TTOT], mybir.dt.float32, kind="ExternalInput")
    idxA_d = nc.dram_tensor("idxA", [P, CIA], mybir.dt.int16, kind="ExternalInput")
    idxB_d = nc.dram_tensor("idxB", [P, CIB], mybir.dt.int16, kind="ExternalInput")
    wdegn_d = nc.dram_tensor("wdegn", [P, NWIN * KD], bf16, kind="ExternalInput")
    iotar_d = nc.dram_tensor("iotar", [P, P], bf16, kind="ExternalInput")
    w1_d = nc.dram_tensor("w1", [F, H], bf16, kind="ExternalInput")
    w2_d = nc.dram_tensor("w2", [H, FO], bf16, kind="ExternalInput")
    b1b_d = nc.dram_tensor("b1b", [P, H], mybir.dt.float32, kind="ExternalInput")
    b2b_d = nc.dram_tensor("b2b", [P, FO], mybir.dt.float32, kind="ExternalInput")
    ident_d = nc.dram_tensor("ident", [P, P], bf16, kind="ExternalInput")
    y_d = nc.dram_tensor("y_win", [SH, FO], mybir.dt.float32, kind="ExternalOutput")

    AF = mybir.ActivationFunctionType
    OP = mybir.AluOpType
    ELLC = 8  # windows per ELL chunk
    EP = 128  # padded t2 row elements (bf16) = 256B

    mctr = [0]

    def m_engine_l1(wi, k):
        # Pool shares L1 M builds only for windows before the chunk-A
        # collective (which blocks Pool once it issues)
        if wi < WA and k % 2 == 0:
            return nc.gpsimd
        return nc.vector

    def m_engine_l2():
        mctr[0] += 1
        if M_POOL_MOD and mctr[0] % M_POOL_MOD == 0:
            return nc.gpsimd
        return nc.vector

    with tile.TileContext(nc) as tc:
        with (
            tc.tile_pool(name="const", bufs=1) as cpool,
            tc.tile_pool(name="ellp", bufs=2) as ellp,
            tc.tile_pool(name="xep", bufs=4) as xep,
            tc.tile_pool(name="mpool", bufs=16) as mpool,
            tc.tile_pool(name="g2p", bufs=12) as g2p,
            tc.tile_pool(name="post", bufs=4) as post,
            tc.tile_pool(name="pacc", bufs=2, space="PSUM") as pacc,
            tc.tile_pool(name="pacc2", bufs=1, space="PSUM") as pacc2,
            tc.tile_pool(name="phT", bufs=2, space="PSUM") as phT,
            tc.tile_pool(name="ph", bufs=2, space="PSUM") as ph,
            tc.tile_pool(name="pmisc", bufs=1, space="PSUM") as pmisc,
            tc.tile_pool(name="dram", bufs=1, space="DRAM") as dpool,
        ):
            for rep in range(reps):
                # ---- constants ----
                iotar_t = cpool.tile([P, P], bf16, tag="iotar")
                nc.sync.dma_start(out=iotar_t[:], in_=iotar_d[:])
                ident_t = cpool.tile([P, P], bf16, tag="ident")
                nc.sync.dma_start(out=ident_t[:], in_=ident_d[:])
                w1_t = cpool.tile([F, H], bf16, tag="w1")
                nc.sync.dma_start(out=w1_t[:], in_=w1_d[:])
                w2_t = cpool.tile([H, FO], bf16, tag="w2")
                nc.sync.dma_start(out=w2_t[:], in_=w2_d[:])
                b1b_t = cpool.tile([P, H], mybir.dt.float32, tag="b1b")
                nc.sync.dma_start(out=b1b_t[:], in_=b1b_d[:])
                b2b_t = cpool.tile([P, FO], mybir.dt.float32, tag="b2b")
                nc.sync.dma_start(out=b2b_t[:], in_=b2b_d[:])
                slots_t = cpool.tile([P, TTOT], mybir.dt.float32, tag="slots")
                nc.sync.dma_start(out=slots_t[:], in_=slots_d[:])
                wraw_t = cpool.tile([P, TTOT], mybir.dt.float32, tag="wraw")
                nc.sync.dma_start(out=wraw_t[:], in_=wraw_d[:])
                idxA_t = cpool.tile([P, CIA], mybir.dt.int16, tag="idxA")
                nc.sync.dma_start(out=idxA_t[:], in_=idxA_d[:])
                idxB_t = cpool.tile([P, CIB], mybir.dt.int16, tag="idxB")
                nc.sync.dma_start(out=idxB_t[:], in_=idxB_d[:])

                # ---- dinv for own nodes [P, NWIN] ----
                wdegn_t = cpool.tile([P, NWIN * KD], bf16, tag="wdegn")
                nc.scalar.dma_start(out=wdegn_t[:], in_=wdegn_d[:])
                degn_t = cpool.tile([P, NWIN], mybir.dt.float32, tag="degn")
                nc.vector.tensor_reduce(
                    out=degn_t[:],
                    in_=wdegn_t[:].rearrange("p (w k) -> p w k", k=KD),
                    axis=mybir.AxisListType.X,
                    op=OP.add,
                )
                nc.vector.tensor_scalar_add(out=degn_t[:], in0=degn_t[:], scalar1=1.0)
                recn_t = cpool.tile([P, NWIN], mybir.dt.float32, tag="recn")
                nc.vector.reciprocal(out=recn_t[:], in_=degn_t[:])
                dinv_t = cpool.tile([P, NWIN], mybir.dt.float32, tag="dinv")
                nc.scalar.activation(out=dinv_t[:], in_=recn_t[:], func=AF.Sqrt)

                # ---- per-edge dinv -> wsc [P, TTOT] (prologue; Sqrt table
                # loads once before L1's Exp) ----
                dege_t = cpool.tile([P, TTOT], mybir.dt.float32, tag="dege")
                rece_t = cpool.tile([P, TTOT], mybir.dt.float32, tag="rece")
                dinve_t = cpool.tile([P, TTOT], mybir.dt.float32, tag="dinve")
                wsc_t = cpool.tile([P, TTOT], mybir.dt.float32, tag="wsc")
                nch = (NWIN + ELLC - 1) // ELLC
                for ci in range(nch):
                    t0 = ci * ELLC * TT
                    t1 = min(TTOT, (ci + 1) * ELLC * TT)
                    el_t = ellp.tile([P, (t1 - t0) * KD], bf16, tag="el")
                    nc.scalar.dma_start(out=el_t[:], in_=ell_d[:, t0 * KD : t1 * KD])
                    ve = nc.vector if ci % 2 == 0 else nc.gpsimd
                    nc.vector.tensor_reduce(
                        out=dege_t[:, t0:t1],
                        in_=el_t[:].rearrange("p (t k) -> p t k", k=KD),
                        axis=mybir.AxisListType.X,
                        op=OP.add,
                    )
                    ve.tensor_scalar_add(
                        out=dege_t[:, t0:t1], in0=dege_t[:, t0:t1], scalar1=1.0
                    )
                    nc.vector.reciprocal(out=rece_t[:, t0:t1], in_=dege_t[:, t0:t1])
                    nc.scalar.activation(
                        out=dinve_t[:, t0:t1], in_=rece_t[:, t0:t1], func=AF.Sqrt
                    )
                    ve.tensor_tensor(
                        out=wsc_t[:, t0:t1], in0=wraw_t[:, t0:t1],
                        in1=dinve_t[:, t0:t1], op=OP.mult,
                    )

                # ---- t2 staging (padded bf16 rows) ----
                t2sbA = cpool.tile([P, WA * EP], bf16, tag="t2sbA")
                nc.gpsimd.memset(t2sbA[:], 0.0)
                t2sbB = cpool.tile([P, WB * EP], bf16, tag="t2sbB")
                nc.gpsimd.memset(t2sbB[:], 0.0)
                t2_shardA = dpool.tile([WA * P, EP], bf16, tag="t2shA")
                t2_shardB = dpool.tile([WB * P, EP], bf16, tag="t2shB")
                t2_fullA = dpool.tile(
                    [RA, EP], bf16, addr_space="Shared", tag=f"t2fA{rep}"
                )
                t2_fullB = dpool.tile(
                    [RB, EP], bf16, addr_space="Shared", tag=f"t2fB{rep}"
                )

                # ---- layer 1 + t2 build; AllGather A fires mid-loop ----
                for wi in range(NWIN):
                    if True:
                        xe_t = xep.tile([P, TT * F], bf16, tag="xe")
                        nc.sync.dma_start(
                            out=xe_t[:], in_=xe_d[wi * P : (wi + 1) * P, :]
                        )
                        xoff = 0
                        acc1 = pacc.tile([F, P], mybir.dt.float32, space="PSUM", tag="acc1")
                        for k in range(TT):
                            t = wi * TT + k
                            m_t = mpool.tile([P, P], bf16, tag="m")
                            m_engine_l1(wi, k).tensor_scalar(
                                out=m_t[:], in0=iotar_t[:],
                                scalar1=slots_t[:, t : t + 1],
                                scalar2=wsc_t[:, t : t + 1],
                                op0=OP.is_equal, op1=OP.mult,
                            )
                            nc.tensor.matmul(
                                out=acc1[:],
                                lhsT=xe_t[:, xoff + k * F : xoff + (k + 1) * F],
                                rhs=m_t[:],
                                start=(k == 0),
                                stop=(k == TT - 1),
                            )
                        acc1_t = post.tile([F, P], bf16, tag="acc1s")
                        nc.scalar.copy(out=acc1_t[:], in_=acc1[:])

                        h_ps = ph.tile([P, H], mybir.dt.float32, space="PSUM", tag="h")
                        nc.tensor.matmul(
                            out=h_ps[:], lhsT=acc1_t[:], rhs=w1_t[:],
                            start=True, stop=True,
                        )
                        e2_t = post.tile([P, H], mybir.dt.float32, tag="e2")
                        nc.vector.scalar_tensor_tensor(
                            out=e2_t[:], in0=h_ps[:], scalar=dinv_t[:, wi : wi + 1],
                            in1=b1b_t[:], op0=OP.mult, op1=OP.add,
                        )
                        mn_t = post.tile([P, H], mybir.dt.float32, tag="mn")
                        nc.vector.tensor_scalar_min(out=mn_t[:], in0=e2_t[:], scalar1=0.0)
                        ex_t = post.tile([P, H], mybir.dt.float32, tag="ex")
                        nc.scalar.activation(out=ex_t[:], in_=mn_t[:], func=AF.Exp)
                        rl_t = post.tile([P, H], mybir.dt.float32, tag="rl")
                        nc.vector.tensor_scalar_max(out=rl_t[:], in0=e2_t[:], scalar1=0.0)
                        h_t = post.tile([P, H], bf16, tag="hf")
                        nc.vector.scalar_tensor_tensor(
                            out=h_t[:], in0=ex_t[:], scalar=-1.0, in1=rl_t[:],
                            op0=OP.add, op1=OP.add,
                        )
                        hT_ps = phT.tile([H, P], bf16, space="PSUM", tag="hT")
                        nc.tensor.transpose(out=hT_ps[:], in_=h_t[:], identity=ident_t[:])
                        hT_t = post.tile([H, P], bf16, tag="hTs")
                        nc.scalar.copy(out=hT_t[:], in_=hT_ps[:])
                        t2_ps = pmisc.tile([P, FO], mybir.dt.float32, space="PSUM", tag="t2")
                        nc.tensor.matmul(
                            out=t2_ps[:], lhsT=hT_t[:], rhs=w2_t[:],
                            start=True, stop=True,
                        )
                        if wi < WA:
                            dstc = t2sbA[:, wi * EP : wi * EP + FO]
                        else:
                            dstc = t2sbB[:, (wi - WA) * EP : (wi - WA) * EP + FO]
                        nc.scalar.mul(out=dstc, in_=t2_ps[:], mul=dinv_t[:, wi : wi + 1])

                        if wi == WA - 1:
                            nc.sync.dma_start(
                                out=t2_shardA[:].rearrange("(w p) f -> p w f", p=P),
                                in_=t2sbA[:].rearrange("p (w f) -> p w f", f=EP),
                            )
                            if KVARIANT != "l1":
                                nc.gpsimd.collective_compute(
                                    "AllGather",
                                    OP.bypass,
                                    replica_groups=[list(range(NCORES))],
                                    ins=[t2_shardA.opt()],
                                    outs=[t2_fullA.opt()],
                                )
                # ---- stage chunk B shard early (input for ccB below) ----
                nc.sync.dma_start(
                    out=t2_shardB[:].rearrange("(w p) f -> p w f", p=P),
                    in_=t2sbB[:].rearrange("p (w f) -> p w f", f=EP),
                )

                # ---- layer 2: pass A (emitted BEFORE ccB so its gathers and
                # Pool M builds run while chunk B is still collecting) ----
                c3a_t = cpool.tile([P, NWIN * FO], mybir.dt.float32, tag="c3a")
                naa_t = cpool.tile([P, NWIN * FO], mybir.dt.float32, tag="naa")
                part_t = cpool.tile([P, NWIN * FO], mybir.dt.float32, tag="partA")

                def l2_pass(pi, tcnt, tstart, tfull, idx_t):
                    if KVARIANT in ("l1cc", "l1"):
                        return
                    ntile_tot = NWIN * tcnt
                    NG = (ntile_tot + 7) // 8
                    gtiles = []

                    def emit_gather(g):
                        th = min(8, ntile_tot - g * 8)
                        ni = th * P
                        gt = g2p.tile([P, 8 * EP], bf16, tag=f"g2{pi}")
                        nc.gpsimd.dma_gather(
                            out_ap=gt[:, : th * EP].rearrange(
                                "p (t f) -> p t f", f=EP
                            ),
                            in_ap=tfull[:],
                            idxs_ap=idx_t[:, g * 64 : g * 64 + th * 8],
                            num_idxs=ni,
                            num_idxs_reg=ni,
                            elem_size=EP,
                            queue_num=1 + g % 3,
                        )
                        gtiles.append(gt)

                    if KVARIANT == "l1g":
                        for g in range(NG):
                            emit_gather(g)
                        return

                    for w in range(NWIN):
                        acc2 = pacc2.tile([P, FO], mybir.dt.float32, space="PSUM", tag="acc2")
                        for t in range(tcnt):
                            gt_idx = w * tcnt + t
                            g, slot = divmod(gt_idx, 8)
                            while len(gtiles) <= min(g + 8, NG - 1):
                                emit_gather(len(gtiles))
                            col = w * TT + tstart + t
                            m2_t = mpool.tile([P, P], bf16, tag="m2")
                            m_engine_l2().tensor_scalar(
                                out=m2_t[:], in0=iotar_t[:],
                                scalar1=slots_t[:, col : col + 1],
                                scalar2=wraw_t[:, col : col + 1],
                                op0=OP.is_equal, op1=OP.mult,
                            )
                            nc.tensor.matmul(
                                out=acc2[:],
                                lhsT=m2_t[:],
                                rhs=gtiles[g][:, slot * EP : slot * EP + FO],
                                start=(t == 0),
                                stop=(t == tcnt - 1),
                            )
                        if pi == 0:
                            nc.scalar.copy(
                                out=part_t[:, w * FO : (w + 1) * FO], in_=acc2[:]
                            )
                        else:
                            sum_t = post.tile([P, FO], mybir.dt.float32, tag="sum")
                            nc.vector.tensor_tensor(
                                out=sum_t[:], in0=acc2[:],
                                in1=part_t[:, w * FO : (w + 1) * FO], op=OP.add,
                            )
                            nc.vector.scalar_tensor_tensor(
                                out=c3a_t[:, w * FO : (w + 1) * FO], in0=sum_t[:],
                                scalar=dinv_t[:, w : w + 1],
                                in1=b2b_t[:], op0=OP.mult, op1=OP.add,
                            )
                            nc.vector.scalar_tensor_tensor(
                                out=naa_t[:, w * FO : (w + 1) * FO],
                                in0=c3a_t[:, w * FO : (w + 1) * FO], scalar=-1.0,
                                in1=c3a_t[:, w * FO : (w + 1) * FO],
                                op0=OP.mult, op1=OP.min,
                            )

                l2_pass(0, TA, 0, t2_fullA, idxA_t)

                # ---- AllGather chunk B (Pool reaches this after pass A's
                # Pool work; COLLECTIVE_CORES free since ccA finished) ----
                if KVARIANT != "l1":
                    nc.gpsimd.collective_compute(
                        "AllGather",
                        OP.bypass,
                        replica_groups=[list(range(NCORES))],
                        ins=[t2_shardB.opt()],
                        outs=[t2_fullB.opt()],
                    )

                l2_pass(1, TB, TA, t2_fullB, idxB_t)

                # ---- bulk softplus: y = max(c3,0) + ln(1+exp(-|c3|)) + 1e-4
                nc.scalar.activation(out=naa_t[:], in_=naa_t[:], func=AF.Exp)
                nc.scalar.activation(out=naa_t[:], in_=naa_t[:], func=AF.Ln, bias=1.0)
                nc.vector.scalar_tensor_tensor(
                    out=c3a_t[:], in0=c3a_t[:], scalar=0.0, in1=naa_t[:],
                    op0=OP.max, op1=OP.add,
                )
                nc.vector.tensor_scalar_add(out=c3a_t[:], in0=c3a_t[:], scalar1=1e-4)
                nc.sync.dma_start(
                    out=y_d[:].rearrange("(w p) f -> p w f", p=P),
                    in_=c3a_t[:].rearrange("p (w f) -> p w f", f=FO),
                )

    nc.compile()
    return nc, in_maps, row_of_node


def kernel(x, edge_index, edge_weight, W1, b1, W2, b2):
    import time

    from concourse.bass_utils import run_bass_kernel_spmd

    nc, in_maps, row_of_node = build_problem(
        x, edge_index, edge_weight, W1, b1, W2, b2
    )
    last_err = None
    for attempt in range(3):
        try:
            res = run_bass_kernel_spmd(nc, in_maps, core_ids=list(range(NCORES)))
            break
        except Exception as e:
            last_err = e
            try:
                import jax

                jax.clear_caches()
            except Exception:
                pass
            time.sleep(30 * (attempt + 1))
    else:
        raise last_err
    y_full = np.concatenate([res.results[c]["y_win"] for c in range(NCORES)], axis=0)
    return y_full[row_of_node].astype(np.float32)


# revision 38
# speedup vs baseline: 2.3862x; 2.3862x over previous
"""Dir_Encoder_GCN v4: bf16 streaming, chunked AllGather overlap, dma_gather.

Per core (nodes dst-sharded; self-loops materialized as edges, weight 1):

- Windows of <=128 nodes and <=TT*128 edges; TT=18 tiles/window split as
  tiles 0..8 = edges whose SOURCE lies in chunk A (source windows < WA on
  the source's core), tiles 9..17 = chunk-B sources.
- L1: x rows streamed in edge order (bf16, one descriptor per partition
  row); matmul(lhsT=x_tile, rhs=M) accumulates [F, slot] in PSUM; W1
  applies from it. M = one-hot(slot) * w_e * dinv[src] built by
  is_equal+mult from a bf16 iota (2x DVE mode); a fraction of M builds
  runs on the Pool engine (tunable).
- dinv[src] per edge from a bf16 ELL of source in-edge weights, reduced +
  rsqrt in a prologue (activation tables load once: Sqrt -> Exp -> Ln).
- t2 = dinv*(h@W2) rows PADDED to 128 bf16 (=256B, dma_gather alignment),
  staged per chunk; AllGather chunk A fires mid-L1 (overlaps remaining L1
  windows), chunk B at L1 end (overlaps L2 pass A). Two tables keep
  row indices < 32k for int16 dma_gather indices.
- L2 two passes (A-tiles then B-tiles): dma_gather in 1024-index chunks
  (8 tiles) decoupled from windows, lazily emitted ahead of use; pass A
  stashes partial sums, pass B combines; softplus finishes in bulk.

build_problem(..., reps=N) repeats the body N times inside one NEFF for
slope-based timing.
"""

import os
import sys

if "/opt/trn_rl_repo" not in sys.path:
    sys.path.insert(0, "/opt/trn_rl_repo")

import numpy as np

N_NODES = 50000
NCORES = 8
P = 128
TA = 9
TB = 9
TT = TA + TB  # 18
M_POOL_MOD = int(os.environ.get("M_POOL_MOD", "0"))  # every k-th M build on Pool; 0=off
KVARIANT = os.environ.get("KVARIANT", "full")  # full | l1cc | l1g (perf bisection)


def _pack2(cA, cB, margin=64):
    """Greedy contiguous windows: <=128 nodes, class caps minus a margin
    (the margin absorbs chunk-flag drift across packing iterations)."""
    windows = []
    lo = 0
    curA = 0
    curB = 0
    capA = TA * P - margin
    capB = TB * P - margin
    n = len(cA)
    hi = 0
    while hi < n:
        a, b = cA[hi], cB[hi]
        if (hi - lo) >= P or curA + a > capA or curB + b > capB:
            windows.append((lo, hi))
            lo = hi
            curA = 0
            curB = 0
        curA += a
        curB += b
        hi += 1
    windows.append((lo, hi))
    return windows


def build_problem(x, edge_index, edge_weight, W1, b1, W2, b2, reps=1):
    import concourse.bacc as bacc
    import concourse.tile as tile
    from concourse import bass, mybir

    bf16 = mybir.dt.bfloat16
    np_bf16 = mybir.dt.np(bf16)

    x = np.asarray(x, dtype=np.float32)
    edge_index = np.asarray(edge_index)
    ew = np.asarray(edge_weight, dtype=np.float32)
    W1 = np.asarray(W1, dtype=np.float32)
    b1 = np.asarray(b1, dtype=np.float32)
    W2 = np.asarray(W2, dtype=np.float32)
    b2 = np.asarray(b2, dtype=np.float32)

    n = x.shape[0]
    F = x.shape[1]          # 128
    H = W1.shape[1]         # 128
    FO = W2.shape[1]        # 64
    per_core_n = (n + NCORES - 1) // NCORES

    # real edges sorted by dst (source in-weight lists for the ELL)
    src_r = edge_index[0].astype(np.int64)
    dst_r = edge_index[1].astype(np.int64)
    order_r = np.argsort(dst_r, kind="stable")
    rs, rd, rw = src_r[order_r], dst_r[order_r], ew[order_r]
    r_e0 = np.searchsorted(rd, np.arange(n + 1))
    real_indeg = np.diff(r_e0)
    KD = max(1, int(real_indeg.max()))

    # all edges incl. self-loops (weight 1), sorted by dst
    src = np.concatenate([rs, np.arange(n, dtype=np.int64)])
    dst = np.concatenate([rd, np.arange(n, dtype=np.int64)])
    wgt = np.concatenate([rw, np.ones(n, dtype=np.float32)])
    order = np.argsort(dst, kind="stable")
    s_d, d_d, w_d = src[order], dst[order], wgt[order]
    node_e0 = np.searchsorted(d_d, np.arange(n + 1))
    indeg = np.diff(node_e0)

    core_lims = [(c * per_core_n, min((c + 1) * per_core_n, n)) for c in range(NCORES)]

    # --- window packing with A/B class caps; chunk flags depend on the
    # packing (source window < WA), so iterate with a growing safety
    # margin until the packing satisfies the full caps under the flags it
    # itself induces ---
    cA = indeg // 2
    cB = indeg - cA
    margin = 64
    core_windows = [_pack2(cA[lo:hi], cB[lo:hi], margin) for lo, hi in core_lims]
    for _it in range(20):
        NWIN = max(len(w) for w in core_windows)
        WA = min((NWIN + 1) // 2, 31)
        win_of_node = np.zeros(n, dtype=np.int64)
        for c, (n_lo, _) in enumerate(core_lims):
            for wi, (lo, hi) in enumerate(core_windows[c]):
                win_of_node[n_lo + lo : n_lo + hi] = wi
        isA_it = win_of_node < WA
        cA = np.bincount(d_d[isA_it[s_d]], minlength=n).astype(np.int64)
        cB = indeg - cA
        viol = 0
        for c, (n_lo, n_hi) in enumerate(core_lims):
            for lo, hi in core_windows[c]:
                if (
                    int(cA[n_lo + lo : n_lo + hi].sum()) > TA * P
                    or int(cB[n_lo + lo : n_lo + hi].sum()) > TB * P
                ):
                    viol += 1
        if viol == 0:
            break
        margin += 32
        core_windows = [_pack2(cA[lo:hi], cB[lo:hi], margin) for lo, hi in core_lims]
    else:
        raise RuntimeError("window packing did not converge")
    NWIN = max(len(w) for w in core_windows)
    WA = min((NWIN + 1) // 2, 31)
    WB = NWIN - WA
    assert WB <= 31, (NWIN, WA, WB)
    # final chunk flags from the final packing; verify full caps under them
    win_of_node = np.zeros(n, dtype=np.int64)
    for c, (n_lo, _) in enumerate(core_lims):
        for wi, (lo, hi) in enumerate(core_windows[c]):
            win_of_node[n_lo + lo : n_lo + hi] = wi
    isA_f = win_of_node < WA
    cA = np.bincount(d_d[isA_f[s_d]], minlength=n).astype(np.int64)
    cB = indeg - cA
    for c, (n_lo, n_hi) in enumerate(core_lims):
        for lo, hi in core_windows[c]:
            a = int(cA[n_lo + lo : n_lo + hi].sum())
            b = int(cB[n_lo + lo : n_lo + hi].sum())
            assert a <= TA * P and b <= TB * P and (hi - lo) <= P, (c, lo, hi, a, b)

    SH = NWIN * P
    TTOT = NWIN * TT
    RA = NCORES * WA * P  # chunk-A table rows
    RB = NCORES * WB * P
    assert RA <= 32640 and RB <= 32640, (RA, RB)

    # global rows
    row_of_node = np.zeros(n, dtype=np.int64)   # output rows: c*SH + wi*P + s
    crow_of_node = np.zeros(n, dtype=np.int64)  # chunk-local t2 rows
    win_of_node = np.zeros(n, dtype=np.int64)
    for c, (n_lo, _) in enumerate(core_lims):
        for wi, (lo, hi) in enumerate(core_windows[c]):
            ids = np.arange(lo, hi)
            row_of_node[n_lo + ids] = c * SH + wi * P + (ids - lo)
            win_of_node[n_lo + ids] = wi
            if wi < WA:
                crow_of_node[n_lo + ids] = c * (WA * P) + wi * P + (ids - lo)
            else:
                crow_of_node[n_lo + ids] = c * (WB * P) + (wi - WA) * P + (ids - lo)
    isA = win_of_node < WA

    NTA = NWIN * TA  # flat A tiles per core
    NTB = NWIN * TB
    CIA = NTA * P // 16  # idx cols
    CIB = NTB * P // 16

    xbf = x.astype(np_bf16)
    in_maps = []
    for c, (n_lo, n_hi) in enumerate(core_lims):
        wins = core_windows[c]

        xe = np.zeros((NWIN * P, TT * F), dtype=np_bf16)
        ell = np.zeros((P, TTOT * KD), dtype=np_bf16)
        slots = np.full((P, TTOT), -1.0, dtype=np.float32)
        wraw = np.zeros((P, TTOT), dtype=np.float32)
        idxA_flat = np.zeros(NTA * P, dtype=np.int64)
        idxB_flat = np.zeros(NTB * P, dtype=np.int64)
        wdegn = np.zeros((P, NWIN * KD), dtype=np_bf16)

        for wi, (lo, hi) in enumerate(wins):
            a0 = node_e0[n_lo + lo]
            b0 = node_e0[n_lo + hi]
            wsrc = s_d[a0:b0]
            wslot = (d_d[a0:b0] - n_lo - lo).astype(np.int64)
            ww = w_d[a0:b0]
            eA = isA[wsrc]
            # per-edge tile (t_g global col) and partition (pp)
            t_g = np.zeros(len(wsrc), dtype=np.int64)
            pp = np.zeros(len(wsrc), dtype=np.int64)
            jA = np.nonzero(eA)[0]
            jB = np.nonzero(~eA)[0]
            t_g[jA] = wi * TT + (np.arange(len(jA)) // P)
            pp[jA] = np.arange(len(jA)) % P
            t_g[jB] = wi * TT + TA + (np.arange(len(jB)) // P)
            pp[jB] = np.arange(len(jB)) % P

            slots[pp, t_g] = wslot.astype(np.float32)
            wraw[pp, t_g] = ww
            xe.reshape(NWIN * P, TT, F)[wi * P + pp, t_g - wi * TT] = xbf[wsrc]
            # gather index streams (chunk-local rows)
            tlA = t_g[jA] - wi * TT
            idxA_flat[(wi * TA + tlA) * P + pp[jA]] = crow_of_node[wsrc[jA]]
            tlB = t_g[jB] - wi * TT - TA
            idxB_flat[(wi * TB + tlB) * P + pp[jB]] = crow_of_node[wsrc[jB]]
            # per-edge source real-in-weight ELL
            lens = real_indeg[wsrc]
            tot = int(lens.sum())
            if tot:
                rep = np.repeat(np.arange(len(wsrc)), lens)
                offs = np.arange(tot) - np.repeat(np.cumsum(lens) - lens, lens)
                vals = rw[np.repeat(r_e0[wsrc], lens) + offs]
                ell.reshape(-1)[
                    pp[rep] * (TTOT * KD) + t_g[rep] * KD + offs
                ] = vals
            # own-node real-in-weight ELL for dinv_dst
            ids = np.arange(lo, hi)
            nl = real_indeg[n_lo + ids]
            ntot = int(nl.sum())
            if ntot:
                nrep = np.repeat(ids - lo, nl)
                noffs = np.arange(ntot) - np.repeat(np.cumsum(nl) - nl, nl)
                nvals = rw[np.repeat(r_e0[n_lo + ids], nl) + noffs]
                wdegn.reshape(-1)[
                    nrep * (NWIN * KD) + wi * KD + noffs
                ] = nvals

        assert idxA_flat.max() < RA and idxB_flat.max() < RB
        idxA16 = np.tile(idxA_flat.reshape(-1, 16).T, (8, 1)).astype(np.int16)
        idxB16 = np.tile(idxB_flat.reshape(-1, 16).T, (8, 1)).astype(np.int16)

        in_maps.append(
            {
                "xe": xe,
                "ell": ell,
                "slots": slots,
                "wraw": wraw,
                "idxA": idxA16,
                "idxB": idxB16,
                "wdegn": wdegn,
                "iotar": np.tile(np.arange(P, dtype=np_bf16), (P, 1)),
                "w1": W1.astype(np_bf16),
                "w2": W2.astype(np_bf16),
                "b1b": np.tile(b1[None, :], (P, 1)).astype(np.float32),
                "b2b": np.tile(b2[None, :], (P, 1)).astype(np.float32),
                "ident": np.eye(P, dtype=np_bf16),
            }
        )

    # ---------------- device program ----------------
    nc = bacc.Bacc(
        "TRN2", target_bir_lowering=False, debug=False, num_devices=NCORES,
        num_swdge_queues=4,
    )

    xe_d = nc.dram_tensor("xe", [NWIN * P, TT * F], bf16, kind="ExternalInput")
    ell_d = nc.dram_tensor("ell", [P, TTOT * KD], bf16, kind="ExternalInput")
    slots_d = nc.dram_tensor("slots", [P, TTOT], mybir.dt.float32, kind="ExternalInput")
    wraw_d = nc.dram_tensor("wraw", [P, TTOT], mybir.dt.float32, kind="ExternalInput")
    idxA_d = nc.dram_tensor("idxA", [P, CIA], mybir.dt.int16, kind="ExternalInput")
    idxB_d = nc.dram_tensor("idxB", [P, CIB], mybir.dt.int16, kind="ExternalInput")
    wdegn_d = nc.dram_tensor("wdegn", [P, NWIN * KD], bf16, kind="ExternalInput")
    iotar_d = nc.dram_tensor("iotar", [P, P], bf16, kind="ExternalInput")
    w1_d = nc.dram_tensor("w1", [F, H], bf16, kind="ExternalInput")
    w2_d = nc.dram_tensor("w2", [H, FO], bf16, kind="ExternalInput")
    b1b_d = nc.dram_tensor("b1b", [P, H], mybir.dt.float32, kind="ExternalInput")
    b2b_d = nc.dram_tensor("b2b", [P, FO], mybir.dt.float32, kind="ExternalInput")
    ident_d = nc.dram_tensor("ident", [P, P], bf16, kind="ExternalInput")
    y_d = nc.dram_tensor("y_win", [SH, FO], mybir.dt.float32, kind="ExternalOutput")

    AF = mybir.ActivationFunctionType
    OP = mybir.AluOpType
    ELLC = 8  # windows per ELL chunk
    EP = 128  # padded t2 row elements (bf16) = 256B

    mctr = [0]

    def m_engine_l1(wi, k):
        # Pool shares L1 M builds only for windows before the chunk-A
        # collective (which blocks Pool once it issues)
        if wi < WA and k % 2 == 0:
            return nc.gpsimd
        return nc.vector

    def m_engine_l2():
        mctr[0] += 1
        if M_POOL_MOD and mctr[0] % M_POOL_MOD == 0:
            return nc.gpsimd
        return nc.vector

    with tile.TileContext(nc) as tc:
        with (
            tc.tile_pool(name="const", bufs=1) as cpool,
            tc.tile_pool(name="ellp", bufs=2) as ellp,
            tc.tile_pool(name="xep", bufs=4) as xep,
            tc.tile_pool(name="mpool", bufs=16) as mpool,
            tc.tile_pool(name="g2p", bufs=12) as g2p,
            tc.tile_pool(name="post", bufs=4) as post,
            tc.tile_pool(name="pacc", bufs=2, space="PSUM") as pacc,
            tc.tile_pool(name="ph", bufs=2, space="PSUM") as ph,
            tc.tile_pool(name="pmisc", bufs=1, space="PSUM") as pmisc,
            tc.tile_pool(name="dram", bufs=1, space="DRAM") as dpool,
        ):
            for rep in range(reps):
                # ---- constants ----
                iotar_t = cpool.tile([P, P], bf16, tag="iotar")
                nc.sync.dma_start(out=iotar_t[:], in_=iotar_d[:])
                ident_t = cpool.tile([P, P], bf16, tag="ident")
                nc.sync.dma_start(out=ident_t[:], in_=ident_d[:])
                w1_t = cpool.tile([F, H], bf16, tag="w1")
                nc.sync.dma_start(out=w1_t[:], in_=w1_d[:])
                w2_t = cpool.tile([H, FO], bf16, tag="w2")
                nc.sync.dma_start(out=w2_t[:], in_=w2_d[:])
                b1b_t = cpool.tile([P, H], mybir.dt.float32, tag="b1b")
                nc.sync.dma_start(out=b1b_t[:], in_=b1b_d[:])
                b2b_t = cpool.tile([P, FO], mybir.dt.float32, tag="b2b")
                nc.sync.dma_start(out=b2b_t[:], in_=b2b_d[:])
                slots_t = cpool.tile([P, TTOT], mybir.dt.float32, tag="slots")
                nc.sync.dma_start(out=slots_t[:], in_=slots_d[:])
                wraw_t = cpool.tile([P, TTOT], mybir.dt.float32, tag="wraw")
                nc.sync.dma_start(out=wraw_t[:], in_=wraw_d[:])
                idxA_t = cpool.tile([P, CIA], mybir.dt.int16, tag="idxA")
                nc.sync.dma_start(out=idxA_t[:], in_=idxA_d[:])
                idxB_t = cpool.tile([P, CIB], mybir.dt.int16, tag="idxB")
                nc.sync.dma_start(out=idxB_t[:], in_=idxB_d[:])

                # ---- dinv for own nodes [P, NWIN] ----
                wdegn_t = cpool.tile([P, NWIN * KD], bf16, tag="wdegn")
                nc.scalar.dma_start(out=wdegn_t[:], in_=wdegn_d[:])
                degn_t = cpool.tile([P, NWIN], mybir.dt.float32, tag="degn")
                nc.vector.tensor_reduce(
                    out=degn_t[:],
                    in_=wdegn_t[:].rearrange("p (w k) -> p w k", k=KD),
                    axis=mybir.AxisListType.X,
                    op=OP.add,
                )
                nc.vector.tensor_scalar_add(out=degn_t[:], in0=degn_t[:], scalar1=1.0)
                recn_t = cpool.tile([P, NWIN], mybir.dt.float32, tag="recn")
                nc.vector.reciprocal(out=recn_t[:], in_=degn_t[:])
                dinv_t = cpool.tile([P, NWIN], mybir.dt.float32, tag="dinv")
                nc.scalar.activation(out=dinv_t[:], in_=recn_t[:], func=AF.Sqrt)

                # ---- per-edge dinv -> wsc [P, TTOT] (prologue; Sqrt table
                # loads once before L1's Exp) ----
                dege_t = cpool.tile([P, TTOT], mybir.dt.float32, tag="dege")
                rece_t = cpool.tile([P, TTOT], mybir.dt.float32, tag="rece")
                dinve_t = cpool.tile([P, TTOT], mybir.dt.float32, tag="dinve")
                wsc_t = cpool.tile([P, TTOT], mybir.dt.float32, tag="wsc")
                nch = (NWIN + ELLC - 1) // ELLC
                for ci in range(nch):
                    t0 = ci * ELLC * TT
                    t1 = min(TTOT, (ci + 1) * ELLC * TT)
                    el_t = ellp.tile([P, (t1 - t0) * KD], bf16, tag="el")
                    nc.scalar.dma_start(out=el_t[:], in_=ell_d[:, t0 * KD : t1 * KD])
                    ve = nc.vector if ci % 2 == 0 else nc.gpsimd
                    nc.vector.tensor_reduce(
                        out=dege_t[:, t0:t1],
                        in_=el_t[:].rearrange("p (t k) -> p t k", k=KD),
                        axis=mybir.AxisListType.X,
                        op=OP.add,
                    )
                    ve.tensor_scalar_add(
                        out=dege_t[:, t0:t1], in0=dege_t[:, t0:t1], scalar1=1.0
                    )
                    nc.vector.reciprocal(out=rece_t[:, t0:t1], in_=dege_t[:, t0:t1])
                    nc.scalar.activation(
                        out=dinve_t[:, t0:t1], in_=rece_t[:, t0:t1], func=AF.Sqrt
                    )
                    ve.tensor_tensor(
                        out=wsc_t[:, t0:t1], in0=wraw_t[:, t0:t1],
                        in1=dinve_t[:, t0:t1], op=OP.mult,
                    )

                # ---- t2 staging (padded bf16 rows) ----
                t2sbA = cpool.tile([P, WA * EP], bf16, tag="t2sbA")
                nc.gpsimd.memset(t2sbA[:], 0.0)
                t2sbB = cpool.tile([P, WB * EP], bf16, tag="t2sbB")
                nc.gpsimd.memset(t2sbB[:], 0.0)
                t2_shardA = dpool.tile([WA * P, EP], bf16, tag="t2shA")
                t2_shardB = dpool.tile([WB * P, EP], bf16, tag="t2shB")
                t2_fullA = dpool.tile(
                    [RA, EP], bf16, addr_space="Shared", tag=f"t2fA{rep}"
                )
                t2_fullB = dpool.tile(
                    [RB, EP], bf16, addr_space="Shared", tag=f"t2fB{rep}"
                )

                # ---- layer 1 + t2 build; AllGather A fires mid-loop ----
                for wi in range(NWIN):
                    if True:
                        xe_t = xep.tile([P, TT * F], bf16, tag="xe")
                        nc.sync.dma_start(
                            out=xe_t[:], in_=xe_d[wi * P : (wi + 1) * P, :]
                        )
                        xoff = 0
                        acc1 = pacc.tile([F, P], mybir.dt.float32, space="PSUM", tag="acc1")
                        for k in range(TT):
                            t = wi * TT + k
                            m_t = mpool.tile([P, P], bf16, tag="m")
                            m_engine_l1(wi, k).tensor_scalar(
                                out=m_t[:], in0=iotar_t[:],
                                scalar1=slots_t[:, t : t + 1],
                                scalar2=wsc_t[:, t : t + 1],
                                op0=OP.is_equal, op1=OP.mult,
                            )
                            nc.tensor.matmul(
                                out=acc1[:],
                                lhsT=xe_t[:, xoff + k * F : xoff + (k + 1) * F],
                                rhs=m_t[:],
                                start=(k == 0),
                                stop=(k == TT - 1),
                            )
                        acc1_t = post.tile([F, P], bf16, tag="acc1s")
                        nc.scalar.copy(out=acc1_t[:], in_=acc1[:])

                        h_ps = ph.tile([P, H], mybir.dt.float32, space="PSUM", tag="h")
                        nc.tensor.matmul(
                            out=h_ps[:], lhsT=acc1_t[:], rhs=w1_t[:],
                            start=True, stop=True,
                        )
                        e2_t = post.tile([P, H], mybir.dt.float32, tag="e2")
                        nc.vector.scalar_tensor_tensor(
                            out=e2_t[:], in0=h_ps[:], scalar=dinv_t[:, wi : wi + 1],
                            in1=b1b_t[:], op0=OP.mult, op1=OP.add,
                        )
                        mn_t = post.tile([P, H], mybir.dt.float32, tag="mn")
                        nc.vector.tensor_scalar_min(out=mn_t[:], in0=e2_t[:], scalar1=0.0)
                        ex_t = post.tile([P, H], mybir.dt.float32, tag="ex")
                        nc.scalar.activation(out=ex_t[:], in_=mn_t[:], func=AF.Exp)
                        rl_t = post.tile([P, H], mybir.dt.float32, tag="rl")
                        nc.vector.tensor_scalar_max(out=rl_t[:], in0=e2_t[:], scalar1=0.0)
                        h_t = post.tile([P, H], bf16, tag="hf")
                        nc.vector.scalar_tensor_tensor(
                            out=h_t[:], in0=ex_t[:], scalar=-1.0, in1=rl_t[:],
                            op0=OP.add, op1=OP.add,
                        )
                        hT_ps = pmisc.tile([H, P], bf16, space="PSUM", tag="hT")
                        nc.tensor.transpose(out=hT_ps[:], in_=h_t[:], identity=ident_t[:])
                        hT_t = post.tile([H, P], bf16, tag="hTs")
                        nc.scalar.copy(out=hT_t[:], in_=hT_ps[:])
                        t2_ps = pmisc.tile([P, FO], mybir.dt.float32, space="PSUM", tag="t2")
                        nc.tensor.matmul(
                            out=t2_ps[:], lhsT=hT_t[:], rhs=w2_t[:],
                            start=True, stop=True,
                        )
                        if wi < WA:
                            dstc = t2sbA[:, wi * EP : wi * EP + FO]
                        else:
                            dstc = t2sbB[:, (wi - WA) * EP : (wi - WA) * EP + FO]
                        nc.scalar.mul(out=dstc, in_=t2_ps[:], mul=dinv_t[:, wi : wi + 1])

                        if wi == WA - 1:
                            nc.sync.dma_start(
                                out=t2_shardA[:].rearrange("(w p) f -> p w f", p=P),
                                in_=t2sbA[:].rearrange("p (w f) -> p w f", f=EP),
                            )
                            if KVARIANT != "l1":
                                nc.gpsimd.collective_compute(
                                    "AllGather",
                                    OP.bypass,
                                    replica_groups=[list(range(NCORES))],
                                    ins=[t2_shardA.opt()],
                                    outs=[t2_fullA.opt()],
                                )
                # ---- stage chunk B shard early (input for ccB below) ----
                nc.sync.dma_start(
                    out=t2_shardB[:].rearrange("(w p) f -> p w f", p=P),
                    in_=t2sbB[:].rearrange("p (w f) -> p w f", f=EP),
                )

                # ---- layer 2: pass A (emitted BEFORE ccB so its gathers and
                # Pool M builds run while chunk B is still collecting) ----
                c3a_t = cpool.tile([P, NWIN * FO], mybir.dt.float32, tag="c3a")
                naa_t = cpool.tile([P, NWIN * FO], mybir.dt.float32, tag="naa")
                part_t = cpool.tile([P, NWIN * FO], mybir.dt.float32, tag="partA")

                def l2_pass(pi, tcnt, tstart, tfull, idx_t):
                    if KVARIANT in ("l1cc", "l1"):
                        return
                    ntile_tot = NWIN * tcnt
                    NG = (ntile_tot + 7) // 8
                    gtiles = []

                    def emit_gather(g):
                        th = min(8, ntile_tot - g * 8)
                        ni = th * P
                        gt = g2p.tile([P, 8 * EP], bf16, tag=f"g2{pi}")
                        nc.gpsimd.dma_gather(
                            out_ap=gt[:, : th * EP].rearrange(
                                "p (t f) -> p t f", f=EP
                            ),
                            in_ap=tfull[:],
                            idxs_ap=idx_t[:, g * 64 : g * 64 + th * 8],
                            num_idxs=ni,
                            num_idxs_reg=ni,
                            elem_size=EP,
                            queue_num=1 + g % 3,
                        )
                        gtiles.append(gt)

                    if KVARIANT == "l1g":
                        for g in range(NG):
                            emit_gather(g)
                        return

                    for w in range(NWIN):
                        acc2 = pacc.tile([P, FO], mybir.dt.float32, space="PSUM", tag="acc2")
                        for t in range(tcnt):
                            gt_idx = w * tcnt + t
                            g, slot = divmod(gt_idx, 8)
                            while len(gtiles) <= min(g + 8, NG - 1):
                                emit_gather(len(gtiles))
                            col = w * TT + tstart + t
                            m2_t = mpool.tile([P, P], bf16, tag="m2")
                            m_engine_l2().tensor_scalar(
                                out=m2_t[:], in0=iotar_t[:],
                                scalar1=slots_t[:, col : col + 1],
                                scalar2=wraw_t[:, col : col + 1],
                                op0=OP.is_equal, op1=OP.mult,
                            )
                            nc.tensor.matmul(
                                out=acc2[:],
                                lhsT=m2_t[:],
                                rhs=gtiles[g][:, slot * EP : slot * EP + FO],
                                start=(t == 0),
                                stop=(t == tcnt - 1),
                            )
                        if pi == 0:
                            nc.scalar.copy(
                                out=part_t[:, w * FO : (w + 1) * FO], in_=acc2[:]
                            )
                        else:
                            sum_t = post.tile([P, FO], mybir.dt.float32, tag="sum")
                            nc.vector.tensor_tensor(
                                out=sum_t[:], in0=acc2[:],
                                in1=part_t[:, w * FO : (w + 1) * FO], op=OP.add,
                            )
                            nc.vector.scalar_tensor_tensor(
                                out=c3a_t[:, w * FO : (w + 1) * FO], in0=sum_t[:],
                                scalar=dinv_t[:, w : w + 1],
                                in1=b2b_t[:], op0=OP.mult, op1=OP.add,
                            )
                            nc.vector.scalar_tensor_tensor(
                                out=naa_t[:, w * FO : (w + 1) * FO],
                                in0=c3a_t[:, w * FO : (w + 1) * FO], scalar=-1.0,
                                in1=c3a_t[:, w * FO : (w + 1) * FO],
                                op0=OP.mult, op1=OP.min,
                            )

                l2_pass(0, TA, 0, t2_fullA, idxA_t)

                # ---- AllGather chunk B (Pool reaches this after pass A's
                # Pool work; COLLECTIVE_CORES free since ccA finished) ----
                if KVARIANT != "l1":
                    nc.gpsimd.collective_compute(
                        "AllGather",
                        OP.bypass,
                        replica_groups=[list(range(NCORES))],
                        ins=[t2_shardB.opt()],
                        outs=[t2_fullB.opt()],
                    )

                l2_pass(1, TB, TA, t2_fullB, idxB_t)

                # ---- bulk softplus: y = max(c3,0) + ln(1+exp(-|c3|)) + 1e-4
                nc.scalar.activation(out=naa_t[:], in_=naa_t[:], func=AF.Exp)
                nc.scalar.activation(out=naa_t[:], in_=naa_t[:], func=AF.Ln, bias=1.0)
                nc.vector.scalar_tensor_tensor(
                    out=c3a_t[:], in0=c3a_t[:], scalar=0.0, in1=naa_t[:],
                    op0=OP.max, op1=OP.add,
                )
                nc.vector.tensor_scalar_add(out=c3a_t[:], in0=c3a_t[:], scalar1=1e-4)
                nc.sync.dma_start(
                    out=y_d[:].rearrange("(w p) f -> p w f", p=P),
                    in_=c3a_t[:].rearrange("p (w f) -> p w f", f=FO),
                )

    nc.compile()
    return nc, in_maps, row_of_node


def kernel(x, edge_index, edge_weight, W1, b1, W2, b2):
    import time

    from concourse.bass_utils import run_bass_kernel_spmd

    nc, in_maps, row_of_node = build_problem(
        x, edge_index, edge_weight, W1, b1, W2, b2
    )
    last_err = None
    for attempt in range(3):
        try:
            res = run_bass_kernel_spmd(nc, in_maps, core_ids=list(range(NCORES)))
            break
        except Exception as e:
            last_err = e
            try:
                import jax

                jax.clear_caches()
            except Exception:
                pass
            time.sleep(30 * (attempt + 1))
    else:
        raise last_err
    y_full = np.concatenate([res.results[c]["y_win"] for c in range(NCORES)], axis=0)
    return y_full[row_of_node].astype(np.float32)
